# revision 2
# baseline (speedup 1.0000x reference)
"""Trainium2 Bass kernel for BaselineSAE (top-k sparse autoencoder).

Computes, for x [B=65536, 256], W_enc = W_dec.T [4096, 256], b_pre [256], k=64:
    z        = relu((x - b_pre) @ W_enc.T)          [B, 4096]
    z_sparse = top-64-per-row masking of z           [B, 4096]
    x_hat    = z_sparse @ W_dec.T + b_pre            [B, 256]

Data-parallel across 8 NeuronCores: batch is split into 8 shards of 8192
rows; weights are replicated. Each core runs 64 tiles of 128 rows:
  PE: x transpose -> fp32 encode matmuls -> (later) zs transpose + fp32r decode
  ACT: relu PSUM->SBUF
  DVE: exact top-64 via 8 rounds of max8 + match_replace, then threshold mask
  DMA: stream z / z_sparse / x_hat back to HBM
"""

import numpy as np

INPUT_DIM = 256
HIDDEN_DIM = 4096
B_TOTAL = 65536
N_CORES = 8
ROWS_PER_CORE = B_TOTAL // N_CORES      # 8192
P = 128                                  # partitions / tile rows
N_TILES = ROWS_PER_CORE // P             # 64
K = 64                                   # top-k
N_CHUNK = 512                            # encode matmul free-dim chunk
ENC_CHUNKS = HIDDEN_DIM // N_CHUNK       # 8
DEC_CHUNKS = HIDDEN_DIM // P             # 32

_COMPILED = {}


def _legalize_waits(nc, mybir, bass_rust):
    """This toolchain allows at most one sync wait per instruction; split
    extra waits onto preceding same-engine NOPs."""
    n_split = 0
    for fn in nc.m.functions:
        for bb in fn.blocks:
            new_insts = []
            for inst in bb.instructions:
                si = inst.sync_info
                if si is not None and len(si.on_wait) > 1:
                    waits = list(si.on_wait)
                    for w in waits[:-1]:
                        nop = bass_rust.InstNoOp(
                            name=f"{inst.name}-ws{n_split}", engine=inst.engine
                        )
                        nop.sync_info = mybir.SyncInfo(on_wait=[w], on_update=[])
                        new_insts.append(nop)
                        n_split += 1
                    inst.sync_info = mybir.SyncInfo(
                        on_wait=[waits[-1]], on_update=list(si.on_update)
                    )
                new_insts.append(inst)
            bb.instructions = new_insts
    return n_split


def _build_nc(unroll=2):
    import concourse.bass as bass
    import concourse.mybir as mybir
    import bass_rust
    from concourse.tile import TileContext

    f32 = mybir.dt.float32
    f32r = mybir.dt.float32r

    nc = bass.Bass("TRN2", target_bir_lowering=False)
    xs_in = nc.declare_dram_parameter("xs", [N_TILES, P, INPUT_DIM], f32, isOutput=False)
    wdec_in = nc.declare_dram_parameter("wdec", [2, P, HIDDEN_DIM], f32, isOutput=False)
    wenc_in = nc.declare_dram_parameter("wenc", [DEC_CHUNKS, P, INPUT_DIM], f32, isOutput=False)
    brep_in = nc.declare_dram_parameter("brep", [P, INPUT_DIM], f32, isOutput=False)
    iden_in = nc.declare_dram_parameter("iden", [P, P], f32, isOutput=False)
    z_out = nc.declare_dram_parameter("z", [N_TILES, P, HIDDEN_DIM], f32, isOutput=True)
    zs_out = nc.declare_dram_parameter("zs", [N_TILES, P, HIDDEN_DIM], f32, isOutput=True)
    xh_out = nc.declare_dram_parameter("xh", [N_TILES, P, INPUT_DIM], f32, isOutput=True)

    with TileContext(nc) as tc:
        with tc.tile_pool(name="static", bufs=1) as spool, \
             tc.tile_pool(name="stage", bufs=2) as stage_pool:
            # --- one-time setup: weights resident in SBUF ---
            wdec = [spool.tile([P, HIDDEN_DIM], f32, name=f"wdec{i}", tag=f"wdec{i}")
                    for i in range(2)]
            for i in range(2):
                nc.sync.dma_start(out=wdec[i][:], in_=wdec_in[i])
            wenc_r = [spool.tile([P, INPUT_DIM], f32r, name=f"wenc_r{j}", tag=f"wenc_r{j}")
                      for j in range(DEC_CHUNKS)]
            for j in range(DEC_CHUNKS):
                st = stage_pool.tile([P, INPUT_DIM], f32, tag="wenc_stage")
                nc.sync.dma_start(out=st[:], in_=wenc_in[j])
                nc.vector.tensor_copy(wenc_r[j][:], st[:])
            brep = spool.tile([P, INPUT_DIM], f32)
            nc.sync.dma_start(out=brep[:], in_=brep_in[:])
            iden = spool.tile([P, P], f32)
            nc.sync.dma_start(out=iden[:], in_=iden_in[:])

            with tc.tile_pool(name="work", bufs=2) as pool, \
                 tc.tile_pool(name="zsT_pool", bufs=1) as zsT_pool, \
                 tc.tile_pool(name="psum", bufs=2, space="PSUM") as psum_pool, \
                 tc.tile_pool(name="psum_t", bufs=2, space="PSUM") as psum_t_pool:

                def tile_body(i):
                    x_t = pool.tile([P, INPUT_DIM], f32, name="x_t", tag="x_t")
                    nc.sync.dma_start(out=x_t[:], in_=xs_in[i])
                    xb = pool.tile([P, INPUT_DIM], f32, name="xb", tag="xb")
                    nc.vector.tensor_tensor(out=xb[:], in0=x_t[:], in1=brep[:],
                                            op=mybir.AluOpType.subtract)
                    # transpose x -> xT (2 chunks of 128 input dims)
                    xT = []
                    for c in range(2):
                        pt = psum_t_pool.tile([P, P], f32, name=f"ptx{c}", tag="pt")
                        nc.tensor.transpose(pt[:], xb[:, c * P:(c + 1) * P], iden[:])
                        xt = pool.tile([P, P], f32, name=f"xT{c}", tag=f"xT{c}")
                        nc.scalar.copy(xt[:], pt[:])
                        xT.append(xt)
                    # encode matmuls + relu
                    z_sb = pool.tile([P, HIDDEN_DIM], f32, name="z_sb", tag="z_sb")
                    for n in range(ENC_CHUNKS):
                        pz = psum_pool.tile([P, N_CHUNK], f32, name="pz", tag="pz")
                        sl = slice(n * N_CHUNK, (n + 1) * N_CHUNK)
                        nc.tensor.matmul(pz[:], xT[0][:], wdec[0][:, sl],
                                         start=True, stop=False)
                        nc.tensor.matmul(pz[:], xT[1][:], wdec[1][:, sl],
                                         start=False, stop=True)
                        nc.scalar.activation(z_sb[:, sl], pz[:],
                                             mybir.ActivationFunctionType.Relu)
                    nc.sync.dma_start(out=z_out[i], in_=z_sb[:])
                    # exact top-64: 8 rounds of max8 + match_replace
                    M = pool.tile([P, K], f32, name="M", tag="M")
                    w = pool.tile([P, HIDDEN_DIM], f32, name="w", tag="w")
                    for r in range(8):
                        src = z_sb if r == 0 else w
                        nc.vector.max(out=M[:, 8 * r:8 * r + 8], in_=src[:])
                        nc.vector.match_replace(out=w[:], in_to_replace=M[:, 8 * r:8 * r + 8],
                                                in_values=src[:], imm_value=0.0)
                    # threshold mask & select (T = 64th largest, inclusive)
                    mask = w  # work buffer is dead after the rounds; reuse
                    nc.vector.tensor_scalar(mask[:], z_sb[:], M[:, K - 1:K], None,
                                            op0=mybir.AluOpType.is_ge)
                    zs_sb = pool.tile([P, HIDDEN_DIM], f32, name="zs_sb", tag="zs_sb")
                    nc.vector.tensor_tensor(out=zs_sb[:], in0=z_sb[:], in1=mask[:],
                                            op=mybir.AluOpType.mult)
                    nc.sync.dma_start(out=zs_out[i], in_=zs_sb[:])
                    # decode: transpose zs (32 chunks) then fp32r matmuls
                    pxh = psum_pool.tile([P, INPUT_DIM], f32, name="pxh", tag="pxh")
                    for j in range(DEC_CHUNKS):
                        ptz = psum_t_pool.tile([P, P], f32, name=f"ptz{j}", tag="pt")
                        nc.tensor.transpose(ptz[:], zs_sb[:, j * P:(j + 1) * P], iden[:])
                        zt = zsT_pool.tile([P, P], f32r, name=f"zsT{j}", tag=f"zsT{j}")
                        if j % 2 == 0:
                            nc.scalar.copy(zt[:], ptz[:])
                        else:
                            nc.vector.tensor_copy(zt[:], ptz[:])
                        nc.tensor.matmul(pxh[:], zt[:], wenc_r[j][:],
                                         start=(j == 0), stop=(j == DEC_CHUNKS - 1))
                    xh_sb = pool.tile([P, INPUT_DIM], f32, name="xh_sb", tag="xh_sb")
                    nc.vector.tensor_tensor(out=xh_sb[:], in0=pxh[:], in1=brep[:],
                                            op=mybir.AluOpType.add)
                    nc.sync.dma_start(out=xh_out[i], in_=xh_sb[:])

                if unroll >= N_TILES:
                    for i in range(N_TILES):
                        tile_body(i)
                else:
                    import concourse.bass as _b
                    with tc.For_i(0, N_TILES, unroll) as iv:
                        for u in range(unroll):
                            tile_body(iv + u)

    _legalize_waits(nc, mybir, bass_rust)
    return nc


def _prepare_static_inputs(W_enc, W_dec, b_pre):
    wdec = np.ascontiguousarray(W_dec.reshape(2, P, HIDDEN_DIM).astype(np.float32))
    wenc = np.ascontiguousarray(W_enc.reshape(DEC_CHUNKS, P, INPUT_DIM).astype(np.float32))
    brep = np.ascontiguousarray(
        np.broadcast_to(b_pre.astype(np.float32), (P, INPUT_DIM)))
    iden = np.eye(P, dtype=np.float32)
    return wdec, wenc, brep, iden


def kernel(x, W_enc, W_dec, b_pre, k):
    assert int(k) == K, f"kernel hardcodes k=64, got {k}"
    x = np.asarray(x, dtype=np.float32)
    assert x.shape == (B_TOTAL, INPUT_DIM)

    from concourse.bass_utils import run_bass_kernel_spmd

    if "nc" not in _COMPILED:
        _COMPILED["nc"] = _build_nc()
    nc = _COMPILED["nc"]

    wdec, wenc, brep, iden = _prepare_static_inputs(
        np.asarray(W_enc, np.float32), np.asarray(W_dec, np.float32),
        np.asarray(b_pre, np.float32))
    shards = x.reshape(N_CORES, N_TILES, P, INPUT_DIM)
    in_maps = [
        {"xs": np.ascontiguousarray(shards[c]), "wdec": wdec, "wenc": wenc,
         "brep": brep, "iden": iden}
        for c in range(N_CORES)
    ]
    res = run_bass_kernel_spmd(nc, in_maps, core_ids=list(range(N_CORES)))

    z = np.empty((B_TOTAL, HIDDEN_DIM), np.float32)
    zs = np.empty((B_TOTAL, HIDDEN_DIM), np.float32)
    xh = np.empty((B_TOTAL, INPUT_DIM), np.float32)
    for c in range(N_CORES):
        r = res.results[c]
        z[c * ROWS_PER_CORE:(c + 1) * ROWS_PER_CORE] = r["z"].reshape(ROWS_PER_CORE, HIDDEN_DIM)
        zs[c * ROWS_PER_CORE:(c + 1) * ROWS_PER_CORE] = r["zs"].reshape(ROWS_PER_CORE, HIDDEN_DIM)
        xh[c * ROWS_PER_CORE:(c + 1) * ROWS_PER_CORE] = r["xh"].reshape(ROWS_PER_CORE, INPUT_DIM)
    return (xh, z, zs)
